# revision 20
# baseline (speedup 1.0000x reference)
"""BitLinear (ternary-quantized linear) Trainium2 kernel.

Computes: scale = clip(mean(|w|, axis=1), 1e-5);  w_q = clip(round(w/scale), -1, 1)
          out = x @ (w_q * scale).T
for x [4, 2048, 2048] f32, w [8192, 2048] f32, out [4, 2048, 8192] f32.

Strategy (8 NeuronCores, tensor-parallel over weight rows / out_features):
  - Each core gets a 1024-row shard of w and a full copy of x (fed pre-transposed
    [d_in, tokens] so the contraction dim lands on SBUF partitions; transposition
    is a host-side layout choice only - all arithmetic happens on device).
  - On device: quantize w rows exactly in fp32. The per-row scale uses a
    blocked-512 two-stage reduction, which reproduces bit-for-bit the
    neuronxcc-lowered jnp.mean of the reference, and
    w_q = (w > scale/2) - (w < -scale/2), which equals clip(round(w/scale),-1,1)
    exactly (round is round-half-even).
  - x is cast to bf16 on the scalar engine; the matmul runs in bf16 (ternary
    w_q is exact in bf16) with fp32 PSUM accumulation; the per-o scale is
    applied in the PSUM->SBUF epilogue on the vector engine.
  - w_q is transposed to [d_in, o] layout on the tensor engine during prologue
    gaps; the weight prologue is split in two o-halves with the first token
    slab's first-half matmuls emitted in between, so the PE starts real work
    while the second half is still quantizing.
  - Output stays o-sharded per core; host concatenates.
"""

import os

import numpy as np

B, S, D_IN, D_OUT = 4, 2048, 2048, 8192
T = B * S  # 8192 tokens
N_CORES = 8
O_SHARD = D_OUT // N_CORES  # 1024
EPS = 1e-05

P = 128
KC = D_IN // P  # 16 contraction chunks
N_OT = O_SHARD // P  # 8 o-tiles per core
T_SLAB = 512  # tokens per x slab kept in SBUF
N_SLABS = T // T_SLAB  # 16
TSUB = T_SLAB // P  # 4 psum blocks per slab
OHALF = O_SHARD // 2  # 512, matmul free dim / psum bank

# knobs (env-tunable for experiments)
USE_DMA_CAST = os.environ.get("BL_DMA_CAST", "0") == "1"
EARLY_SLABS = int(os.environ.get("BL_EARLY_SLABS", "0"))

_CACHE = {}


def _build_program():
    import concourse.bass as bass
    import concourse.tile as tile
    from concourse import bacc, mybir
    from concourse.masks import make_identity

    f32 = mybir.dt.float32
    bf16 = mybir.dt.bfloat16

    nc = bacc.Bacc(
        "TRN2",
        target_bir_lowering=False,
        debug=False,
        num_devices=N_CORES,
    )

    xT = nc.dram_tensor("xT", [D_IN, T], f32, kind="ExternalInput")
    w = nc.dram_tensor("w", [O_SHARD, D_IN], f32, kind="ExternalInput")
    out = nc.dram_tensor("out", [T, O_SHARD], f32, kind="ExternalOutput")

    xT3 = xT.ap().rearrange("(c p) t -> p c t", p=P)  # [128, 16, 8192]

    with tile.TileContext(nc) as tc:
        const_pool = tc.alloc_tile_pool(name="const", bufs=1)
        wqt_pool = tc.alloc_tile_pool(name="wq_T", bufs=1)
        sb_pool = tc.alloc_tile_pool(name="scaleB", bufs=1)
        w_pool = tc.alloc_tile_pool(name="wstage", bufs=2)
        wq_pool = tc.alloc_tile_pool(name="wq", bufs=2)
        st_pool = tc.alloc_tile_pool(name="stats", bufs=N_OT)
        psum_pro = tc.alloc_tile_pool(name="psum_pro", bufs=1, space="PSUM")
        xb_pool = tc.alloc_tile_pool(name="xb", bufs=3)
        xf_pool = tc.alloc_tile_pool(name="xf", bufs=3)
        out_pool = tc.alloc_tile_pool(name="osb", bufs=4)
        psum_mm = tc.alloc_tile_pool(name="psum_mm", bufs=2, space="PSUM")
        dram_pool = tc.alloc_tile_pool(name="dram", bufs=1, space="DRAM")
        ctx_pools = [const_pool, wqt_pool, sb_pool, w_pool, wq_pool, st_pool,
                     psum_pro, xb_pool, xf_pool, out_pool, psum_mm, dram_pool]

        ident_bf = const_pool.tile([P, P], bf16)
        make_identity(nc, ident_bf)
        ident_f32 = const_pool.tile([P, P], f32)
        make_identity(nc, ident_f32)
        ones_f32 = const_pool.tile([P, P], f32)
        nc.vector.memset(ones_f32[:], 1.0)

        # resident: transposed ternary weights (one tile per o-half) and
        # the per-o scale broadcast across all 128 partitions
        wqTh = [wqt_pool.tile([P, KC, OHALF], bf16, tag=f"wqT{h}",
                              name=f"wqT{h}")
                for h in range(2)]
        scaleB = sb_pool.tile([P, O_SHARD], f32)
        wq_dram = dram_pool.tile([O_SHARD, D_IN], bf16)
        # ds_full[p, ot*128+j] = scale[ot*128+p] * I[p, j]: every column has
        # exactly one nonzero, so ones.T @ ds_full == scale broadcast over
        # all partitions
        ds_full = sb_pool.tile([P, O_SHARD], f32)

        def emit_scale_broadcast():
            for h in range(2):
                bp = psum_pro.tile([P, OHALF], f32, tag="bp", name="bp",
                                   bufs=1)
                nc.tensor.matmul(bp[:], ones_f32[:],
                                 ds_full[:, bass.ts(h, OHALF)],
                                 start=True, stop=True)
                nc.scalar.copy(out=scaleB[:, bass.ts(h, OHALF)], in_=bp[:])

        def prologue_otile(ot, via_dma=False):
            """Quantize o-tile `ot` of w and transpose it into wqTh."""
            wf = w_pool.tile([P, D_IN], f32, name="wf")
            nc.sync.dma_start(wf[:], w[bass.ts(ot, P), :])

            # blocked-512 two-stage reduce: bit-exact match with the
            # neuronxcc-lowered jnp.mean the reference runs through
            ssum4 = st_pool.tile([P, 4], f32, tag="ssum4", name="ssum4")
            nc.vector.tensor_reduce(
                out=ssum4[:],
                in_=wf[:].rearrange("p (b k) -> p b k", k=512),
                op=mybir.AluOpType.add,
                axis=mybir.AxisListType.X,
                apply_absolute_value=True,
            )
            ssum = st_pool.tile([P, 1], f32, tag="ssum", name="ssum")
            nc.vector.tensor_reduce(
                out=ssum[:], in_=ssum4[:],
                op=mybir.AluOpType.add, axis=mybir.AxisListType.X,
            )
            # thr = scale/2 and nthr = -scale/2 computed directly from ssum;
            # 0.5/D_IN is an exact power-of-two scaling and HEPS = fl32(EPS)/2
            # is exact, so these match max(clip(mean),EPS)/2 bit-for-bit
            HEPS = float(np.float32(EPS)) * 0.5
            thr = st_pool.tile([P, 1], f32, tag="thr", name="thr")
            nc.vector.tensor_scalar(
                thr[:], ssum[:], 0.5 / D_IN, HEPS,
                mybir.AluOpType.mult, mybir.AluOpType.max,
            )
            nthr = st_pool.tile([P, 1], f32, tag="nthr", name="nthr")
            nc.vector.tensor_scalar(
                nthr[:], ssum[:], -0.5 / D_IN, -HEPS,
                mybir.AluOpType.mult, mybir.AluOpType.min,
            )
            scale = st_pool.tile([P, 1], f32, tag="scale", name="scale")
            nc.vector.tensor_scalar(
                scale[:], ssum[:], 1.0 / D_IN, EPS,
                mybir.AluOpType.mult, mybir.AluOpType.max,
            )

            # w_q = (w > thr) - (w < -thr)  in {-1, 0, 1}, exact in bf16
            neg = wq_pool.tile([P, D_IN], f32, tag="neg", name="neg")
            nc.vector.tensor_scalar(
                neg[:], wf[:], nthr[:], None, mybir.AluOpType.is_lt,
            )
            wq = wq_pool.tile([P, D_IN], bf16, tag="wq", name="wq")
            nc.vector.scalar_tensor_tensor(
                out=wq[:], in0=wf[:], scalar=thr[:], in1=neg[:],
                op0=mybir.AluOpType.is_gt, op1=mybir.AluOpType.subtract,
            )

            # transpose wq [o,i] -> wqT [i,o]
            h, col = divmod(ot * P, OHALF)
            if via_dma:
                # park wq in DRAM; the xbar transpose reads it back later
                nc.sync.dma_start(wq_dram[bass.ts(ot, P), :], wq[:])
            else:
                for kc in range(KC):
                    pt = psum_pro.tile([P, P], bf16, tag="tp", name="pt",
                                       bufs=3)
                    nc.tensor.transpose(pt[:], wq[:, bass.ts(kc, P)],
                                        ident_bf[:])
                    nc.scalar.copy(out=wqTh[h][:, kc, bass.ds(col, P)],
                                   in_=pt[:])

            # per-tile diagonal block of ds_full (used by the broadcast
            # matmuls emitted after the prologue)
            nc.vector.tensor_scalar(
                ds_full[:, bass.ts(ot, P)], ident_f32[:], scale[:], None,
                mybir.AluOpType.mult,
            )

        xb_tiles = {}

        def load_slab(s):
            tsl = bass.ts(s, T_SLAB)
            xb = xb_pool.tile([P, KC, T_SLAB], bf16, name="xb")
            if USE_DMA_CAST:
                nc.gpsimd.dma_start(xb[:], xT3[:, :, tsl])
            else:
                for q in range(4):
                    xf = xf_pool.tile([P, KC // 4, T_SLAB], f32, name="xf")
                    nc.sync.dma_start(xf[:], xT3[:, bass.ts(q, KC // 4), tsl])
                    nc.scalar.copy(out=xb[:, bass.ts(q, KC // 4), :], in_=xf[:])
            xb_tiles[s] = xb

        osb_tiles = {}

        def mm_half(s, tsub, h):
            """Matmuls + scale epilogue for one 128-token block, one o-half."""
            xb = xb_tiles[s]
            ps = psum_mm.tile([P, OHALF], mybir.dt.float32, tag=f"ps{h}",
                              name="ps")
            for kc in range(KC):
                nc.tensor.matmul(
                    ps[:],
                    xb[:, kc, bass.ts(tsub, P)],
                    wqTh[h][:, kc, :],
                    start=(kc == 0),
                    stop=(kc == KC - 1),
                )
            _epilogue(s, tsub, h, ps)

        def _epilogue(s, tsub, h, ps):
            osb = out_pool.tile([P, OHALF], f32, tag=f"osb{h}", name="osb")
            nc.vector.tensor_tensor(
                osb[:], ps[:], scaleB[:, bass.ts(h, OHALF)],
                mybir.AluOpType.mult,
            )
            row0 = (s * TSUB + tsub) * P
            nc.sync.dma_start(
                out[bass.ds(row0, P), bass.ts(h, OHALF)], osb[:]
            )

        def mm_block(s, tsub):
            """Both o-halves of one 128-token block with a single LDWEIGHTS
            per x tile (halves interleaved inside the kc loop)."""
            xb = xb_tiles[s]
            pss = [psum_mm.tile([P, OHALF], mybir.dt.float32, tag=f"ps{h}",
                                name="ps")
                   for h in range(2)]
            for kc in range(KC):
                lhsT = xb[:, kc, bass.ts(tsub, P)]
                for h in range(2):
                    nc.tensor.matmul(
                        pss[h][:], lhsT, wqTh[h][:, kc, :],
                        start=(kc == 0), stop=(kc == KC - 1),
                    )
            for h in range(2):
                _epilogue(s, tsub, h, pss[h])

        def store_block(s, tsub):
            pass

        # ---------------- emission schedule -----------------------------
        # Half 0 of w transposes on the PE (hidden under the DVE quant of
        # half 1); half 1 goes through DRAM + the DMA xbar transpose, which
        # overlaps the first slabs' half-0 matmuls. All PE prologue ops
        # stay strictly before the first matmul (interleaving PE transposes
        # between matmul groups faults the hardware).
        nE = max(0, min(EARLY_SLABS, N_SLABS))
        if nE == 0:
            for ot in range(N_OT):
                prologue_otile(ot)
            emit_scale_broadcast()
            for s in range(N_SLABS):
                load_slab(s)
                for tsub in range(TSUB):
                    mm_block(s, tsub)
        else:
            # interleave: half-0 prologue, early half-0 matmuls, half-1
            # prologue, rest. PE drains flush the LDWEIGHTS reorder window
            # at every transpose-mode <-> matmul-mode transition (the
            # window otherwise pulls a transpose-mode LDWEIGHTS ahead of
            # in-flight matmuls, which faults the exec unit).
            for s in range(nE):
                load_slab(s)
            for ot in range(N_OT // 2):
                prologue_otile(ot)
            nc.tensor.drain()
            for s in range(nE):
                for tsub in range(TSUB):
                    mm_half(s, tsub, 0)
            nc.tensor.drain()
            for ot in range(N_OT // 2, N_OT):
                prologue_otile(ot)
            nc.tensor.drain()
            for s in range(nE):
                for tsub in range(TSUB):
                    mm_half(s, tsub, 1)
                    store_block(s, tsub)
            for s in range(nE, N_SLABS):
                load_slab(s)
                for tsub in range(TSUB):
                    mm_half(s, tsub, 0)
                    mm_half(s, tsub, 1)
                    store_block(s, tsub)

        for p in reversed(ctx_pools):
            p.release()

    nc.compile()
    return nc


def _get_program():
    if "nc" not in _CACHE:
        _CACHE["nc"] = _build_program()
    return _CACHE["nc"]


def _ensure_ntff_hook():
    """Provide antenv.axon_hooks if the image lacks it (profiling only)."""
    import sys
    import types

    try:
        from antenv.axon_hooks import get_axon_ntff_profile_hook  # noqa: F401
        return
    except ImportError:
        pass
    try:
        import antenv
        from trn_agent_boot.trn_boot import _ntff_profile_via_ctypes

        mod = types.ModuleType("antenv.axon_hooks")
        state = {"hook": _ntff_profile_via_ctypes("/opt/axon/libaxon_pjrt.so")}
        mod.get_axon_ntff_profile_hook = lambda: state["hook"]
        mod.set_axon_ntff_profile_hook = lambda h: state.__setitem__("hook", h)
        sys.modules["antenv.axon_hooks"] = mod
        antenv.axon_hooks = mod
    except Exception:
        pass


def kernel(x: np.ndarray, weight: np.ndarray) -> np.ndarray:
    from concourse.bass_utils import run_bass_kernel_spmd

    assert x.shape == (B, S, D_IN) and weight.shape == (D_OUT, D_IN)
    nc = _get_program()

    xT = np.ascontiguousarray(x.reshape(T, D_IN).T)
    in_maps = [
        {"xT": xT, "w": weight[c * O_SHARD : (c + 1) * O_SHARD]}
        for c in range(N_CORES)
    ]

    trace = os.environ.get("BL_TRACE", "0") == "1"
    if trace:
        _ensure_ntff_hook()
    res = run_bass_kernel_spmd(nc, in_maps, list(range(N_CORES)), trace=trace)
    _CACHE["last_results"] = res

    parts = [res.results[c]["out"] for c in range(N_CORES)]
    full = np.concatenate(parts, axis=1)  # [T, D_OUT]
    return np.ascontiguousarray(full.reshape(B, S, D_OUT)).astype(np.float32, copy=False)


# revision 21
# speedup vs baseline: 1.0170x; 1.0170x over previous
"""BitLinear (ternary-quantized linear) Trainium2 kernel.

Computes: scale = clip(mean(|w|, axis=1), 1e-5);  w_q = clip(round(w/scale), -1, 1)
          out = x @ (w_q * scale).T
for x [4, 2048, 2048] f32, w [8192, 2048] f32, out [4, 2048, 8192] f32.

Strategy (8 NeuronCores, tensor-parallel over weight rows / out_features):
  - Each core gets a 1024-row shard of w and a full copy of x (fed pre-transposed
    [d_in, tokens] so the contraction dim lands on SBUF partitions; transposition
    is a host-side layout choice only - all arithmetic happens on device).
  - On device: quantize w rows exactly in fp32. The per-row scale uses a
    blocked-512 two-stage reduction, which reproduces bit-for-bit the
    neuronxcc-lowered jnp.mean of the reference, and
    w_q = (w > scale/2) - (w < -scale/2), which equals clip(round(w/scale),-1,1)
    exactly (round is round-half-even).
  - x is cast to bf16 on the scalar engine; the matmul runs in bf16 (ternary
    w_q is exact in bf16) with fp32 PSUM accumulation; the per-o scale is
    applied in the PSUM->SBUF epilogue on the vector engine.
  - w_q is transposed to [d_in, o] layout on the tensor engine during prologue
    gaps; the weight prologue is split in two o-halves with the first token
    slab's first-half matmuls emitted in between, so the PE starts real work
    while the second half is still quantizing.
  - Output stays o-sharded per core; host concatenates.
"""

import os

import numpy as np

B, S, D_IN, D_OUT = 4, 2048, 2048, 8192
T = B * S  # 8192 tokens
N_CORES = 8
O_SHARD = D_OUT // N_CORES  # 1024
EPS = 1e-05

P = 128
KC = D_IN // P  # 16 contraction chunks
N_OT = O_SHARD // P  # 8 o-tiles per core
T_SLAB = 512  # tokens per x slab kept in SBUF
N_SLABS = T // T_SLAB  # 16
TSUB = T_SLAB // P  # 4 psum blocks per slab
OHALF = O_SHARD // 2  # 512, matmul free dim / psum bank

# knobs (env-tunable for experiments)
USE_DMA_CAST = os.environ.get("BL_DMA_CAST", "0") == "1"
EARLY_SLABS = int(os.environ.get("BL_EARLY_SLABS", "0"))

_CACHE = {}


def _build_program():
    import concourse.bass as bass
    import concourse.tile as tile
    from concourse import bacc, mybir
    from concourse.masks import make_identity

    f32 = mybir.dt.float32
    bf16 = mybir.dt.bfloat16

    nc = bacc.Bacc(
        "TRN2",
        target_bir_lowering=False,
        debug=False,
        num_devices=N_CORES,
    )

    xT = nc.dram_tensor("xT", [D_IN, T], f32, kind="ExternalInput")
    w = nc.dram_tensor("w", [O_SHARD, D_IN], f32, kind="ExternalInput")
    out = nc.dram_tensor("out", [T, O_SHARD], f32, kind="ExternalOutput")

    xT3 = xT.ap().rearrange("(c p) t -> p c t", p=P)  # [128, 16, 8192]

    with tile.TileContext(nc) as tc:
        const_pool = tc.alloc_tile_pool(name="const", bufs=1)
        wqt_pool = tc.alloc_tile_pool(name="wq_T", bufs=1)
        sb_pool = tc.alloc_tile_pool(name="scaleB", bufs=1)
        w_pool = tc.alloc_tile_pool(name="wstage", bufs=2)
        wq_pool = tc.alloc_tile_pool(name="wq", bufs=2)
        st_pool = tc.alloc_tile_pool(name="stats", bufs=N_OT)
        psum_pro = tc.alloc_tile_pool(name="psum_pro", bufs=1, space="PSUM")
        xb_pool = tc.alloc_tile_pool(name="xb", bufs=3)
        xf_pool = tc.alloc_tile_pool(name="xf", bufs=3)
        out_pool = tc.alloc_tile_pool(name="osb", bufs=4)
        psum_mm = tc.alloc_tile_pool(name="psum_mm", bufs=2, space="PSUM")
        dram_pool = tc.alloc_tile_pool(name="dram", bufs=1, space="DRAM")
        ctx_pools = [const_pool, wqt_pool, sb_pool, w_pool, wq_pool, st_pool,
                     psum_pro, xb_pool, xf_pool, out_pool, psum_mm, dram_pool]

        ident_bf = const_pool.tile([P, P], bf16)
        make_identity(nc, ident_bf)
        ident_f32 = const_pool.tile([P, P], f32)
        make_identity(nc, ident_f32)
        ones_f32 = const_pool.tile([P, P], f32)
        nc.vector.memset(ones_f32[:], 1.0)

        # resident: transposed ternary weights (one tile per o-half) and
        # the per-o scale broadcast across all 128 partitions
        wqTh = [wqt_pool.tile([P, KC, OHALF], bf16, tag=f"wqT{h}",
                              name=f"wqT{h}")
                for h in range(2)]
        scaleB = sb_pool.tile([P, O_SHARD], f32)
        wq_dram = dram_pool.tile([O_SHARD, D_IN], bf16)

        def prologue_otile(ot, via_dma=False):
            """Quantize o-tile `ot` of w and transpose it into wqTh."""
            wf = w_pool.tile([P, D_IN], f32, name="wf")
            nc.sync.dma_start(wf[:], w[bass.ts(ot, P), :])

            # blocked-512 two-stage reduce: bit-exact match with the
            # neuronxcc-lowered jnp.mean the reference runs through
            ssum4 = st_pool.tile([P, 4], f32, tag="ssum4", name="ssum4")
            nc.vector.tensor_reduce(
                out=ssum4[:],
                in_=wf[:].rearrange("p (b k) -> p b k", k=512),
                op=mybir.AluOpType.add,
                axis=mybir.AxisListType.X,
                apply_absolute_value=True,
            )
            ssum = st_pool.tile([P, 1], f32, tag="ssum", name="ssum")
            nc.vector.tensor_reduce(
                out=ssum[:], in_=ssum4[:],
                op=mybir.AluOpType.add, axis=mybir.AxisListType.X,
            )
            scale = st_pool.tile([P, 1], f32, tag="scale", name="scale")
            nc.vector.tensor_scalar(
                scale[:], ssum[:], 1.0 / D_IN, EPS,
                mybir.AluOpType.mult, mybir.AluOpType.max,
            )
            thr = st_pool.tile([P, 1], f32, tag="thr", name="thr")
            nc.vector.tensor_scalar_mul(thr[:], scale[:], 0.5)
            nthr = st_pool.tile([P, 1], f32, tag="nthr", name="nthr")
            nc.vector.tensor_scalar_mul(nthr[:], thr[:], -1.0)

            # w_q = (w > thr) - (w < -thr)  in {-1, 0, 1}, exact in bf16
            neg = wq_pool.tile([P, D_IN], f32, tag="neg", name="neg")
            nc.vector.tensor_scalar(
                neg[:], wf[:], nthr[:], None, mybir.AluOpType.is_lt,
            )
            wq = wq_pool.tile([P, D_IN], bf16, tag="wq", name="wq")
            nc.vector.scalar_tensor_tensor(
                out=wq[:], in0=wf[:], scalar=thr[:], in1=neg[:],
                op0=mybir.AluOpType.is_gt, op1=mybir.AluOpType.subtract,
            )

            # transpose wq [o,i] -> wqT [i,o]
            h, col = divmod(ot * P, OHALF)
            if via_dma:
                # park wq in DRAM; the xbar transpose reads it back later
                nc.sync.dma_start(wq_dram[bass.ts(ot, P), :], wq[:])
            else:
                for kc in range(KC):
                    pt = psum_pro.tile([P, P], bf16, tag="tp", name="pt",
                                       bufs=3)
                    nc.tensor.transpose(pt[:], wq[:, bass.ts(kc, P)],
                                        ident_bf[:])
                    nc.scalar.copy(out=wqTh[h][:, kc, bass.ds(col, P)],
                                   in_=pt[:])

            # scaleB[:, ot*128:+128] = scale broadcast over partitions:
            # ones.T @ diag(scale)
            ds_t = wq_pool.tile([P, P], f32, tag="diag", name="ds_t")
            nc.vector.tensor_scalar(
                ds_t[:], ident_f32[:], scale[:], None, mybir.AluOpType.mult,
            )
            bp = psum_pro.tile([P, P], f32, tag="bp", name="bp", bufs=1)
            nc.tensor.matmul(bp[:], ones_f32[:], ds_t[:], start=True, stop=True)
            nc.scalar.copy(out=scaleB[:, bass.ts(ot, P)], in_=bp[:])

        xb_tiles = {}

        def load_slab(s):
            tsl = bass.ts(s, T_SLAB)
            xb = xb_pool.tile([P, KC, T_SLAB], bf16, name="xb")
            if USE_DMA_CAST:
                nc.gpsimd.dma_start(xb[:], xT3[:, :, tsl])
            else:
                for q in range(4):
                    xf = xf_pool.tile([P, KC // 4, T_SLAB], f32, name="xf")
                    nc.sync.dma_start(xf[:], xT3[:, bass.ts(q, KC // 4), tsl])
                    nc.scalar.copy(out=xb[:, bass.ts(q, KC // 4), :], in_=xf[:])
            xb_tiles[s] = xb

        osb_tiles = {}

        def mm_half(s, tsub, h):
            """Matmuls + scale epilogue for one 128-token block, one o-half."""
            xb = xb_tiles[s]
            ps = psum_mm.tile([P, OHALF], mybir.dt.float32, tag=f"ps{h}",
                              name="ps")
            for kc in range(KC):
                nc.tensor.matmul(
                    ps[:],
                    xb[:, kc, bass.ts(tsub, P)],
                    wqTh[h][:, kc, :],
                    start=(kc == 0),
                    stop=(kc == KC - 1),
                )
            osb = out_pool.tile([P, OHALF], f32, tag=f"osb{h}", name="osb")
            nc.vector.tensor_tensor(
                osb[:], ps[:], scaleB[:, bass.ts(h, OHALF)],
                mybir.AluOpType.mult,
            )
            row0 = (s * TSUB + tsub) * P
            nc.sync.dma_start(
                out[bass.ds(row0, P), bass.ts(h, OHALF)], osb[:]
            )

        def store_block(s, tsub):
            pass

        # ---------------- emission schedule -----------------------------
        # Half 0 of w transposes on the PE (hidden under the DVE quant of
        # half 1); half 1 goes through DRAM + the DMA xbar transpose, which
        # overlaps the first slabs' half-0 matmuls. All PE prologue ops
        # stay strictly before the first matmul (interleaving PE transposes
        # between matmul groups faults the hardware).
        nE = max(0, min(EARLY_SLABS, N_SLABS))
        if nE == 0:
            for ot in range(N_OT):
                prologue_otile(ot)
            for s in range(N_SLABS):
                load_slab(s)
                for tsub in range(TSUB):
                    mm_half(s, tsub, 0)
                    mm_half(s, tsub, 1)
                    store_block(s, tsub)
        else:
            # interleave: half-0 prologue, early half-0 matmuls, half-1
            # prologue, rest. PE drains flush the LDWEIGHTS reorder window
            # at every transpose-mode <-> matmul-mode transition (the
            # window otherwise pulls a transpose-mode LDWEIGHTS ahead of
            # in-flight matmuls, which faults the exec unit).
            for s in range(nE):
                load_slab(s)
            for ot in range(N_OT // 2):
                prologue_otile(ot)
            nc.tensor.drain()
            for s in range(nE):
                for tsub in range(TSUB):
                    mm_half(s, tsub, 0)
            nc.tensor.drain()
            for ot in range(N_OT // 2, N_OT):
                prologue_otile(ot)
            nc.tensor.drain()
            for s in range(nE):
                for tsub in range(TSUB):
                    mm_half(s, tsub, 1)
                    store_block(s, tsub)
            for s in range(nE, N_SLABS):
                load_slab(s)
                for tsub in range(TSUB):
                    mm_half(s, tsub, 0)
                    mm_half(s, tsub, 1)
                    store_block(s, tsub)

        for p in reversed(ctx_pools):
            p.release()

    nc.compile()
    return nc


def _get_program():
    if "nc" not in _CACHE:
        _CACHE["nc"] = _build_program()
    return _CACHE["nc"]


def _ensure_ntff_hook():
    """Provide antenv.axon_hooks if the image lacks it (profiling only)."""
    import sys
    import types

    try:
        from antenv.axon_hooks import get_axon_ntff_profile_hook  # noqa: F401
        return
    except ImportError:
        pass
    try:
        import antenv
        from trn_agent_boot.trn_boot import _ntff_profile_via_ctypes

        mod = types.ModuleType("antenv.axon_hooks")
        state = {"hook": _ntff_profile_via_ctypes("/opt/axon/libaxon_pjrt.so")}
        mod.get_axon_ntff_profile_hook = lambda: state["hook"]
        mod.set_axon_ntff_profile_hook = lambda h: state.__setitem__("hook", h)
        sys.modules["antenv.axon_hooks"] = mod
        antenv.axon_hooks = mod
    except Exception:
        pass


def kernel(x: np.ndarray, weight: np.ndarray) -> np.ndarray:
    from concourse.bass_utils import run_bass_kernel_spmd

    assert x.shape == (B, S, D_IN) and weight.shape == (D_OUT, D_IN)
    nc = _get_program()

    xT = np.ascontiguousarray(x.reshape(T, D_IN).T)
    in_maps = [
        {"xT": xT, "w": weight[c * O_SHARD : (c + 1) * O_SHARD]}
        for c in range(N_CORES)
    ]

    trace = os.environ.get("BL_TRACE", "0") == "1"
    if trace:
        _ensure_ntff_hook()
    res = run_bass_kernel_spmd(nc, in_maps, list(range(N_CORES)), trace=trace)
    _CACHE["last_results"] = res

    parts = [res.results[c]["out"] for c in range(N_CORES)]
    full = np.concatenate(parts, axis=1)  # [T, D_OUT]
    return np.ascontiguousarray(full.reshape(B, S, D_OUT)).astype(np.float32, copy=False)
